# revision 31
# baseline (speedup 1.0000x reference)
"""Trainium2 Bass kernel for the DigitConvolutionalModel problem.

Math: out = relu(conv3x3(x) @ fc1_w.T + fc1_b) @ fc2_w.T + fc2_b
The 3x3 valid conv followed by a dense layer composes into a single
linear map, so conv_w and fc1_w are folded on the host into one
W1eff [128, 784] matrix. The device then runs two matmuls + bias/relu.

Sharding: pure data parallelism - batch split across 8 cores.

Precision: single fp16 products everywhere (x, W1eff, h, W2 all fp16;
PSUM accumulates f32). End-to-end rel max err ~5e-4 vs the 2e-2 gate.
This halves HBM traffic vs an fp16 hi+lo compensated scheme and cuts
fc1 to 7 matmuls per 512-chunk, so the kernel sits right at the
DMA roofline (~13 MB per core at ~360 GB/s).

Layout: x is staged per-core as a flat [128, 6*8192] fp16 tensor made
of per-block permuted slabs: block k (n_k columns starting at global
column G_k) occupies flat cols [6*G_k, 6*G_k + 6*n_k) as [chunk c][col
j] with element = x_t[c*128 + p, G_k + j]. Each block is then one
128-descriptor DMA with 12*n_k contiguous bytes per partition. The 16
leftover K rows (768:784) ship once as a [16, 8192] tail tensor.

Schedule: block sizes decrease geometrically (ratio ~ PE-rate / bus-
rate = 0.78) so the PE finishes each block just as the next lands and
the final block is tiny; the final block additionally arrives as six
K-chunk pieces so the very last arrival gates exactly one matmul.

PE p-state: matmul cost is locked in near dependency-ready time; if
the PE is idle or <3us into a busy run at that moment the matmul pays
a 2-3.7x slower rate forever. Warmup matmuls on a memset tile keep
the PE busy (and its ramp running) until the first block lands.
"""

import numpy as np

import concourse.bacc as bacc
import concourse.mybir as mybir
import concourse.tile as tile
from concourse.bass_utils import run_bass_kernel_spmd

N_CORES = 8
B = 65536
B_LOCAL = B // N_CORES  # 8192
K = 784                 # input features (28*28)
KM = 768                # main K rows (6 chunks of 128)
KT = 16                 # tail rows
M1 = 128                # fc1 out
M2 = 10                 # fc2 out
NKC = 6                 # main K chunks

F32 = mybir.dt.float32
FP16 = mybir.dt.float16

NS = 512                # matmul moving-dim subtile (one PSUM bank)

# geometric delivery schedule (sums to B_LOCAL, last block stays small)
BLOCKS = [1024, 1024, 1024, 1024, 1024, 1024, 768, 512, 512, 256]
assert sum(BLOCKS) == B_LOCAL

N_WARM = 10             # warmup matmuls before the first real chunk
ZB_QUEUE = "sync"       # engine issuing the final z DMA
FINAL_SWAP = False      # final chunk: relu on DVE, bias-add on ACT

_cache = {}


def _build_nc():
    nc = bacc.Bacc("TRN2", target_bir_lowering=False, debug=False,
                   num_devices=N_CORES)

    x_d = nc.dram_tensor("x_p", [128, NKC * B_LOCAL], FP16,
                         kind="ExternalInput")
    xt_d = nc.dram_tensor("x_tail", [KT, B_LOCAL], FP16,
                          kind="ExternalInput")
    # cols 0:768 = six [128,128] W1 chunks; rows 0:16 of 768:896 = tail
    # weight; 896:906 = W2
    w_d = nc.dram_tensor("w_pack", [128, 906], FP16, kind="ExternalInput")
    # col 0 = b1, col 1 rows 0:10 = b2
    b_d = nc.dram_tensor("b_pack", [128, 2], F32, kind="ExternalInput")
    z_d = nc.dram_tensor("z_t", [M2, B_LOCAL], FP16, kind="ExternalOutput")

    nblk = len(BLOCKS)
    goffs = [sum(BLOCKS[:k]) for k in range(nblk)]
    n_last = BLOCKS[-1]

    with tile.TileContext(nc) as tc:
        with (
            tc.tile_pool(name="static", bufs=1) as sp,
            tc.tile_pool(name="xp", bufs=1) as xp,
            tc.tile_pool(name="hp", bufs=4) as hp,
            # z accumulates in two one-shot tiles (no recycling: their DMAs
            # sit behind the whole x stream in the shared-bus FIFO)
            tc.tile_pool(name="zp", bufs=1) as zp,
            tc.tile_pool(name="pp1", bufs=3, space="PSUM") as pp1,
            tc.tile_pool(name="pp2", bufs=2, space="PSUM") as pp2,
            tc.tile_pool(name="ppd", bufs=1, space="PSUM") as ppd,
        ):
            # warmup operand needs no DMA: memset lets the PE start matmuls
            # almost immediately, so its 3us p-state ramp completes before
            # any real matmul's cost is locked in
            wu = sp.tile([128, NS], FP16, tag="wu")
            nc.vector.memset(wu[:], 1.0)

            # small static loads ride the SP HWDGE queue so their descriptor
            # generation overlaps the x-stream generation on GPSIMD SWDGE
            w = sp.tile([128, 906], FP16, tag="w")
            nc.sync.dma_start(w[:], w_d[:])
            b = sp.tile([128, 2], F32, tag="b")
            nc.sync.dma_start(b[:], b_d[:])
            xtail = sp.tile([KT, B_LOCAL], FP16, tag="xtail")
            nc.sync.dma_start(xtail[:], xt_d[:])

            b1 = b[:, 0:1]
            b2 = b[0:M2, 1:2]
            w1cs = [w[:, c * 128:(c + 1) * 128] for c in range(NKC)]
            wtl = w[0:KT, 768:896]
            w2 = w[:, 896:906]

            # x stream: all blocks issued up front (they all fit in SBUF);
            # bus serialization does the pacing. The final block arrives as
            # six separate K-chunk piece tiles.
            xts = []
            xfine = []
            for k in range(nblk):
                g6 = NKC * goffs[k]
                n = BLOCKS[k]
                if k == nblk - 1:
                    for c in range(NKC):
                        xf = xp.tile([128, n], FP16, tag=f"xf{c}")
                        nc.gpsimd.dma_start(
                            xf[:], x_d[:, g6 + c * n:g6 + (c + 1) * n])
                        xfine.append(xf)
                    xts.append(None)
                else:
                    xt_t = xp.tile([128, NKC, n], FP16, tag=f"x{k}")
                    nc.gpsimd.dma_start(xt_t[:],
                                        x_d[:, g6:g6 + NKC * n])
                    xts.append(xt_t)

            # PE warmup: keep the tensor engine busy (and its p-state
            # ramped) on junk matmuls while the first block streams in.
            psd = ppd.tile([M1, NS], F32, tag="psd")
            for _ in range(N_WARM):
                nc.tensor.matmul(psd[:], wu[:, 0:128], wu[:],
                                 start=True, stop=True, skip_group_check=True)

            # fc2 of chunk i is deferred until after chunk i+1's fc1 so the
            # PE never waits on ACT's h output. z accumulates in two SBUF
            # tiles: all but the last block go out in one DMA right behind
            # the last x transfer (so no z slice ever delays the x stream on
            # the shared bus); the last block's z goes out the moment it is
            # ready.
            zA = zp.tile([M2, B_LOCAL - n_last], FP16, tag="zA")
            zB = zp.tile([M2, n_last], FP16, tag="zB")
            pending = []

            def flush_pending():
                for h_t, zt_t, lo, n, dma, fin in pending:
                    ps2 = pp2.tile([M2, NS], F32, tag="ps2")
                    nc.tensor.matmul(ps2[:, 0:n], w2, h_t,
                                     start=True, stop=True)
                    if fin and FINAL_SWAP:
                        nc.scalar.activation(
                            zt_t[:, lo:lo + n], ps2[:, 0:n],
                            mybir.ActivationFunctionType.Identity, bias=b2)
                    else:
                        nc.vector.tensor_scalar_add(
                            zt_t[:, lo:lo + n], ps2[:, 0:n], b2)
                    if dma is not None:
                        eng = getattr(nc, ZB_QUEUE) if fin else nc.sync
                        eng.dma_start(*dma)
                pending.clear()

            ps_fine = [None]

            def emit_fine_head(n):
                # first 6 matmuls of the final block, interleaved into the
                # previous block's stream so only c5 remains at the end
                pf = pp1.tile([M1, NS], F32, tag="ps1")
                for c in range(NKC - 1):
                    nc.tensor.matmul(pf[:, 0:n], w1cs[c], xfine[c][:],
                                     start=(c == 0), stop=False)
                nc.tensor.matmul(pf[:, 0:n], wtl,
                                 xtail[:, goffs[-1]:goffs[-1] + n],
                                 start=False, stop=False)
                ps_fine[0] = pf

            for k in range(nblk):
                last = k == nblk - 1
                n_blk = BLOCKS[k]
                csched = [NS] * (n_blk // NS)
                if n_blk % NS:
                    csched.append(n_blk % NS)
                off = 0
                for ci, n in enumerate(csched):
                    goff = goffs[k] + off
                    if last:
                        # last-arriving piece (c5) is the stop matmul
                        ps1 = ps_fine[0]
                        nc.tensor.matmul(ps1[:, 0:n], w1cs[NKC - 1],
                                         xfine[NKC - 1][:],
                                         start=False, stop=True)
                    else:
                        ps1 = pp1.tile([M1, NS], F32, tag="ps1")
                        for c in range(NKC):
                            nc.tensor.matmul(
                                ps1[:, 0:n], w1cs[c],
                                xts[k][:, c, off:off + n],
                                start=(c == 0), stop=False)
                        nc.tensor.matmul(ps1[:, 0:n], wtl,
                                         xtail[:, goff:goff + n],
                                         start=False, stop=True)
                        if k == nblk - 2 and ci == len(csched) - 1:
                            emit_fine_head(n_last)
                    h = hp.tile([M1, NS], FP16, tag="h")
                    if last and FINAL_SWAP:
                        nc.vector.tensor_scalar(
                            h[:, 0:n], ps1[:, 0:n], b1, 0.0,
                            mybir.AluOpType.add, mybir.AluOpType.max)
                    else:
                        nc.scalar.activation(
                            h[:, 0:n], ps1[:, 0:n],
                            mybir.ActivationFunctionType.Relu, bias=b1)
                    flush_pending()
                    final_chunk = ci == len(csched) - 1
                    if last:
                        pending.append((h[:, 0:n], zB, off, n,
                                        (z_d[:, goffs[k]:], zB[:]), True))
                    else:
                        dma = ((z_d[:, 0:B_LOCAL - n_last], zA[:])
                               if k == nblk - 2 and final_chunk else None)
                        pending.append((h[:, 0:n], zA, goff, n, dma, False))
                    off += n
            flush_pending()
    nc.compile()
    return nc


def _fold_weights(conv_w, fc1_w):
    """Fold 3x3 valid cross-correlation + fc1 into one [128, 784] matrix."""
    cw = np.asarray(conv_w, np.float64)
    f1 = np.asarray(fc1_w, np.float64).reshape(M1, 26, 26)
    W = np.zeros((M1, 28, 28), np.float64)
    for di in range(3):
        for dj in range(3):
            W[:, di:di + 26, dj:dj + 26] += cw[di, dj] * f1
    return W.reshape(M1, K).astype(np.float32)


def kernel(x, conv_w, fc1_w, fc1_b, fc2_w, fc2_b):
    if "nc" not in _cache:
        _cache["nc"] = _build_nc()
    nc = _cache["nc"]

    w1t = _fold_weights(conv_w, fc1_w).T.astype(np.float16)  # [784, 128]
    w_pack = np.zeros((128, 906), np.float16)
    for c in range(NKC):
        w_pack[:, c * 128:(c + 1) * 128] = w1t[c * 128:(c + 1) * 128, :]
    w_pack[0:KT, 768:896] = w1t[KM:K, :]
    w_pack[:, 896:906] = np.asarray(fc2_w, np.float32).T.astype(np.float16)
    b_pack = np.zeros((128, 2), np.float32)
    b_pack[:, 0] = np.asarray(fc1_b, np.float32)
    b_pack[0:M2, 1] = np.asarray(fc2_b, np.float32)

    x = np.asarray(x, np.float32)
    in_maps = []
    for cid in range(N_CORES):
        xs = x[cid * B_LOCAL:(cid + 1) * B_LOCAL].T.astype(np.float16)
        # flat per-block permuted slabs: block k (cols G:G+n) ->
        # flat[:, 6G + c*n + j] = xs[c*128 + p, G + j]
        xm = np.empty((128, NKC * B_LOCAL), np.float16)
        g = 0
        for n in BLOCKS:
            blk = xs[:KM, g:g + n].reshape(NKC, 128, n)
            xm[:, NKC * g:NKC * (g + n)] = (
                blk.transpose(1, 0, 2).reshape(128, NKC * n))
            g += n
        in_maps.append({
            "x_p": xm,
            "x_tail": np.ascontiguousarray(xs[KM:K]),
            "w_pack": w_pack, "b_pack": b_pack,
        })
    res = run_bass_kernel_spmd(nc, in_maps, list(range(N_CORES)))
    outs = [res.results[c]["z_t"].T for c in range(N_CORES)]
    return np.ascontiguousarray(
        np.concatenate(outs, axis=0).astype(np.float32))


# revision 34
# speedup vs baseline: 1.0116x; 1.0116x over previous
"""Trainium2 Bass kernel for the DigitConvolutionalModel problem.

Math: out = relu(conv3x3(x) @ fc1_w.T + fc1_b) @ fc2_w.T + fc2_b
The 3x3 valid conv followed by a dense layer composes into a single
linear map, so conv_w and fc1_w are folded on the host into one
W1eff [128, 784] matrix. The device then runs two matmuls + bias/relu.

Sharding: pure data parallelism - batch split across 8 cores.

Precision: single fp16 products everywhere (x, W1eff, h, W2 all fp16;
PSUM accumulates f32). End-to-end rel max err ~5e-4 vs the 2e-2 gate.
This halves HBM traffic vs an fp16 hi+lo compensated scheme and cuts
fc1 to 7 matmuls per 512-chunk, so the kernel sits right at the
DMA roofline (~13 MB per core at ~360 GB/s).

Layout: x is staged per-core as a flat [128, 6*8192] fp16 tensor made
of per-block permuted slabs: block k (n_k columns starting at global
column G_k) occupies flat cols [6*G_k, 6*G_k + 6*n_k) as [chunk c][col
j] with element = x_t[c*128 + p, G_k + j]. Each block is then one
128-descriptor DMA with 12*n_k contiguous bytes per partition. The 16
leftover K rows (768:784) ship once as a [16, 8192] tail tensor.

Schedule: block sizes decrease geometrically (ratio ~ PE-rate / bus-
rate = 0.78) so the PE finishes each block just as the next lands and
the final block is tiny; the final block additionally arrives as six
K-chunk pieces so the very last arrival gates exactly one matmul.

PE p-state: matmul cost is locked in near dependency-ready time; if
the PE is idle or <3us into a busy run at that moment the matmul pays
a 2-3.7x slower rate forever. Warmup matmuls on a memset tile keep
the PE busy (and its ramp running) until the first block lands.
"""

import numpy as np

import concourse.bacc as bacc
import concourse.mybir as mybir
import concourse.tile as tile
from concourse.bass_utils import run_bass_kernel_spmd

N_CORES = 8
B = 65536
B_LOCAL = B // N_CORES  # 8192
K = 784                 # input features (28*28)
KM = 768                # main K rows (6 chunks of 128)
KT = 16                 # tail rows
M1 = 128                # fc1 out
M2 = 10                 # fc2 out
NKC = 6                 # main K chunks

F32 = mybir.dt.float32
FP16 = mybir.dt.float16

NS = 512                # matmul moving-dim subtile (one PSUM bank)

# geometric delivery schedule (sums to B_LOCAL, last block stays small)
BLOCKS = [1256, 1024, 1024, 1024, 1024, 800, 624, 488, 376, 296, 256]
assert sum(BLOCKS) == B_LOCAL

N_WARM = 10             # warmup matmuls before the first real chunk
ZB_QUEUE = "sync"       # engine issuing the final z DMA
DEFER_DEPTH = 2         # chunks of fc2 deferral behind fc1
FINAL_SWAP = False      # final chunk: relu on DVE, bias-add on ACT

_cache = {}


def _build_nc():
    nc = bacc.Bacc("TRN2", target_bir_lowering=False, debug=False,
                   num_devices=N_CORES)

    x_d = nc.dram_tensor("x_p", [128, NKC * B_LOCAL], FP16,
                         kind="ExternalInput")
    xt_d = nc.dram_tensor("x_tail", [KT, B_LOCAL], FP16,
                          kind="ExternalInput")
    # cols 0:768 = six [128,128] W1 chunks; rows 0:16 of 768:896 = tail
    # weight; 896:906 = W2
    w_d = nc.dram_tensor("w_pack", [128, 906], FP16, kind="ExternalInput")
    # col 0 = b1, col 1 rows 0:10 = b2
    b_d = nc.dram_tensor("b_pack", [128, 2], F32, kind="ExternalInput")
    z_d = nc.dram_tensor("z_t", [M2, B_LOCAL], FP16, kind="ExternalOutput")

    nblk = len(BLOCKS)
    goffs = [sum(BLOCKS[:k]) for k in range(nblk)]
    n_last = BLOCKS[-1]

    with tile.TileContext(nc) as tc:
        with (
            tc.tile_pool(name="static", bufs=1) as sp,
            tc.tile_pool(name="xp", bufs=1) as xp,
            tc.tile_pool(name="hp", bufs=4) as hp,
            # z accumulates in two one-shot tiles (no recycling: their DMAs
            # sit behind the whole x stream in the shared-bus FIFO)
            tc.tile_pool(name="zp", bufs=1) as zp,
            tc.tile_pool(name="pp1", bufs=3, space="PSUM") as pp1,
            tc.tile_pool(name="pp2", bufs=2, space="PSUM") as pp2,
            tc.tile_pool(name="ppd", bufs=1, space="PSUM") as ppd,
        ):
            # warmup operand needs no DMA: memset lets the PE start matmuls
            # almost immediately, so its 3us p-state ramp completes before
            # any real matmul's cost is locked in
            wu = sp.tile([128, NS], FP16, tag="wu")
            nc.vector.memset(wu[:], 1.0)

            # small static loads ride the SP HWDGE queue so their descriptor
            # generation overlaps the x-stream generation on GPSIMD SWDGE
            w = sp.tile([128, 906], FP16, tag="w")
            nc.sync.dma_start(w[:], w_d[:])
            b = sp.tile([128, 2], F32, tag="b")
            nc.sync.dma_start(b[:], b_d[:])
            xtail = sp.tile([KT, B_LOCAL], FP16, tag="xtail")
            nc.sync.dma_start(xtail[:], xt_d[:])

            b1 = b[:, 0:1]
            b2 = b[0:M2, 1:2]
            w1cs = [w[:, c * 128:(c + 1) * 128] for c in range(NKC)]
            wtl = w[0:KT, 768:896]
            w2 = w[:, 896:906]

            # x stream: all blocks issued up front (they all fit in SBUF);
            # bus serialization does the pacing. The final block arrives as
            # six separate K-chunk piece tiles.
            xts = []
            xfine = []
            for k in range(nblk):
                g6 = NKC * goffs[k]
                n = BLOCKS[k]
                if k == nblk - 1:
                    for c in range(NKC):
                        xf = xp.tile([128, n], FP16, tag=f"xf{c}")
                        nc.gpsimd.dma_start(
                            xf[:], x_d[:, g6 + c * n:g6 + (c + 1) * n])
                        xfine.append(xf)
                    xts.append(None)
                else:
                    xt_t = xp.tile([128, NKC, n], FP16, tag=f"x{k}")
                    nc.gpsimd.dma_start(xt_t[:],
                                        x_d[:, g6:g6 + NKC * n])
                    xts.append(xt_t)

            # PE warmup: keep the tensor engine busy (and its p-state
            # ramped) on junk matmuls while the first block streams in.
            psd = ppd.tile([M1, NS], F32, tag="psd")
            for _ in range(N_WARM):
                nc.tensor.matmul(psd[:], wu[:, 0:128], wu[:],
                                 start=True, stop=True, skip_group_check=True)

            # fc2 of chunk i is deferred until after chunk i+1's fc1 so the
            # PE never waits on ACT's h output. z accumulates in two SBUF
            # tiles: all but the last block go out in one DMA right behind
            # the last x transfer (so no z slice ever delays the x stream on
            # the shared bus); the last block's z goes out the moment it is
            # ready.
            zA = zp.tile([M2, B_LOCAL - n_last], FP16, tag="zA")
            zB = zp.tile([M2, n_last], FP16, tag="zB")
            pending = []

            def flush_pending(keep=0):
                while len(pending) > keep:
                    h_t, zt_t, lo, n, dma, fin = pending.pop(0)
                    ps2 = pp2.tile([M2, NS], F32, tag="ps2")
                    nc.tensor.matmul(ps2[:, 0:n], w2, h_t,
                                     start=True, stop=True)
                    if fin and FINAL_SWAP:
                        nc.scalar.activation(
                            zt_t[:, lo:lo + n], ps2[:, 0:n],
                            mybir.ActivationFunctionType.Identity, bias=b2)
                    else:
                        nc.vector.tensor_scalar_add(
                            zt_t[:, lo:lo + n], ps2[:, 0:n], b2)
                    if dma is not None:
                        eng = getattr(nc, ZB_QUEUE) if fin else nc.sync
                        eng.dma_start(*dma)

            ps_fine = [None]

            def emit_fine_head(n):
                # first 6 matmuls of the final block, interleaved into the
                # previous block's stream so only c5 remains at the end
                pf = pp1.tile([M1, NS], F32, tag="ps1")
                for c in range(NKC - 1):
                    nc.tensor.matmul(pf[:, 0:n], w1cs[c], xfine[c][:],
                                     start=(c == 0), stop=False)
                nc.tensor.matmul(pf[:, 0:n], wtl,
                                 xtail[:, goffs[-1]:goffs[-1] + n],
                                 start=False, stop=False)
                ps_fine[0] = pf

            for k in range(nblk):
                last = k == nblk - 1
                n_blk = BLOCKS[k]
                csched = [NS] * (n_blk // NS)
                if n_blk % NS:
                    csched.append(n_blk % NS)
                off = 0
                for ci, n in enumerate(csched):
                    goff = goffs[k] + off
                    if last:
                        # last-arriving piece (c5) is the stop matmul
                        ps1 = ps_fine[0]
                        nc.tensor.matmul(ps1[:, 0:n], w1cs[NKC - 1],
                                         xfine[NKC - 1][:],
                                         start=False, stop=True)
                    else:
                        ps1 = pp1.tile([M1, NS], F32, tag="ps1")
                        for c in range(NKC):
                            nc.tensor.matmul(
                                ps1[:, 0:n], w1cs[c],
                                xts[k][:, c, off:off + n],
                                start=(c == 0), stop=False)
                        nc.tensor.matmul(ps1[:, 0:n], wtl,
                                         xtail[:, goff:goff + n],
                                         start=False, stop=True)
                        if k == nblk - 2 and ci == len(csched) - 1:
                            emit_fine_head(n_last)
                    h = hp.tile([M1, NS], FP16, tag="h")
                    if last and FINAL_SWAP:
                        nc.vector.tensor_scalar(
                            h[:, 0:n], ps1[:, 0:n], b1, 0.0,
                            mybir.AluOpType.add, mybir.AluOpType.max)
                    else:
                        nc.scalar.activation(
                            h[:, 0:n], ps1[:, 0:n],
                            mybir.ActivationFunctionType.Relu, bias=b1)
                    flush_pending(keep=DEFER_DEPTH - 1)
                    final_chunk = ci == len(csched) - 1
                    if last:
                        pending.append((h[:, 0:n], zB, off, n,
                                        (z_d[:, goffs[k]:], zB[:]), True))
                    else:
                        dma = ((z_d[:, 0:B_LOCAL - n_last], zA[:])
                               if k == nblk - 2 and final_chunk else None)
                        pending.append((h[:, 0:n], zA, goff, n, dma, False))
                    off += n
            flush_pending()
    nc.compile()
    return nc


def _fold_weights(conv_w, fc1_w):
    """Fold 3x3 valid cross-correlation + fc1 into one [128, 784] matrix."""
    cw = np.asarray(conv_w, np.float64)
    f1 = np.asarray(fc1_w, np.float64).reshape(M1, 26, 26)
    W = np.zeros((M1, 28, 28), np.float64)
    for di in range(3):
        for dj in range(3):
            W[:, di:di + 26, dj:dj + 26] += cw[di, dj] * f1
    return W.reshape(M1, K).astype(np.float32)


def kernel(x, conv_w, fc1_w, fc1_b, fc2_w, fc2_b):
    if "nc" not in _cache:
        _cache["nc"] = _build_nc()
    nc = _cache["nc"]

    w1t = _fold_weights(conv_w, fc1_w).T.astype(np.float16)  # [784, 128]
    w_pack = np.zeros((128, 906), np.float16)
    for c in range(NKC):
        w_pack[:, c * 128:(c + 1) * 128] = w1t[c * 128:(c + 1) * 128, :]
    w_pack[0:KT, 768:896] = w1t[KM:K, :]
    w_pack[:, 896:906] = np.asarray(fc2_w, np.float32).T.astype(np.float16)
    b_pack = np.zeros((128, 2), np.float32)
    b_pack[:, 0] = np.asarray(fc1_b, np.float32)
    b_pack[0:M2, 1] = np.asarray(fc2_b, np.float32)

    x = np.asarray(x, np.float32)
    in_maps = []
    for cid in range(N_CORES):
        xs = x[cid * B_LOCAL:(cid + 1) * B_LOCAL].T.astype(np.float16)
        # flat per-block permuted slabs: block k (cols G:G+n) ->
        # flat[:, 6G + c*n + j] = xs[c*128 + p, G + j]
        xm = np.empty((128, NKC * B_LOCAL), np.float16)
        g = 0
        for n in BLOCKS:
            blk = xs[:KM, g:g + n].reshape(NKC, 128, n)
            xm[:, NKC * g:NKC * (g + n)] = (
                blk.transpose(1, 0, 2).reshape(128, NKC * n))
            g += n
        in_maps.append({
            "x_p": xm,
            "x_tail": np.ascontiguousarray(xs[KM:K]),
            "w_pack": w_pack, "b_pack": b_pack,
        })
    res = run_bass_kernel_spmd(nc, in_maps, list(range(N_CORES)))
    outs = [res.results[c]["z_t"].T for c in range(N_CORES)]
    return np.ascontiguousarray(
        np.concatenate(outs, axis=0).astype(np.float32))
